# revision 6
# baseline (speedup 1.0000x reference)
"""Trainium2 Bass kernel for nn_AssociativeMemoryStep (forward-looking retention).

reference semantics:
    q,k,v,o weights = basis @ {q,k,v,o}_coeffs.T          [V, C]
    q/k/v = x @ w                                         [B, T, C]
    scores[t,s] = (q_t . k_s) * decay^(s-t-1) for s>t     (anti-causal)
    retrieved = scores @ v ; out = retrieved @ o_w.T * out_scale

Chunkwise-recurrent backward formulation (state S_t = sum_{s>=t} decay^(s-t) k_s^T v_s):
    retrieved[t] = q_t @ S_{t+1}-style decomposition into intra-chunk masked
    attention + cross-chunk q~ @ S, with S recursed backward per L-chunk.

Sharding: 8 cores = 4 batches x 2 sequence halves of T_loc=2048. Each core gets a
HALO=512 slice of the next half's x and recomputes the boundary state locally
(decay^512 ~ 1.6e-11 truncation), so cores are fully independent (no collectives).

Device layout is fully transposed: qT/kT/rT are [C, T]; v/k~ are [T, C]; the
output is produced as outT [V, T_loc] and transposed back on host.
"""

import numpy as np
import ml_dtypes

import concourse.bass as bass
import concourse.mybir as mybir
import concourse.tile as tile
from concourse import bacc
from concourse.bass_utils import run_bass_kernel_spmd

BF16 = ml_dtypes.bfloat16

B, T, V, C = 4, 4096, 1024, 256
N_CORES = 8
T_LOC = 2048          # main positions per core
HALO = 512            # halo positions (state-only)
T_EXT = T_LOC + HALO
L = 256               # retention chunk
PCH = 512             # projection t-chunk
N_PCH = T_EXT // PCH  # 5 (4 main + 1 halo)
N_MAIN_PCH = T_LOC // PCH  # 4
NCH = T_EXT // L      # 10 L-chunks (8 main + 2 halo)
N_MAIN_CH = T_LOC // L     # 8
KT = V // 128         # 8 v-ktiles
CT = C // 128         # 2 c-tiles
TTILES = T_EXT // 128  # 20 t-tiles (16 main + 4 halo)

FP32 = mybir.dt.float32
BF = mybir.dt.bfloat16


def build_nc():
    nc = bacc.Bacc("TRN2", target_bir_lowering=False, debug=False,
                   num_devices=N_CORES)

    xT_d = nc.dram_tensor("xT", [V, T_EXT], BF, kind="ExternalInput")
    wq_d = nc.dram_tensor("wq", [V, C], BF, kind="ExternalInput")
    wk_d = nc.dram_tensor("wk", [V, C], BF, kind="ExternalInput")
    wv_d = nc.dram_tensor("wv", [V, C], BF, kind="ExternalInput")
    owT_d = nc.dram_tensor("owT", [C, V], BF, kind="ExternalInput")
    maskT_d = nc.dram_tensor("maskT", [L, L], FP32, kind="ExternalInput")
    crossb_d = nc.dram_tensor("crossb", [128, PCH], FP32, kind="ExternalInput")
    kscale_d = nc.dram_tensor("kscale", [128, 2], FP32, kind="ExternalInput")
    ident_d = nc.dram_tensor("ident", [128, 128], BF, kind="ExternalInput")
    dident_d = nc.dram_tensor("dident", [128, 128], BF, kind="ExternalInput")
    outT_d = nc.dram_tensor("outT", [V, T_LOC], BF, kind="ExternalOutput")

    with tile.TileContext(nc) as tc:
        build_tile(tc, xT_d, wq_d, wk_d, wv_d, owT_d, maskT_d, crossb_d,
                   kscale_d, ident_d, dident_d, outT_d)
    nc.compile()
    return nc


def build_tile(tc, xT_d, wq_d, wk_d, wv_d, owT_d, maskT_d, crossb_d,
               kscale_d, ident_d, dident_d, outT_d):
    nc = tc.nc

    import contextlib
    ctx = contextlib.ExitStack()
    consts = ctx.enter_context(tc.tile_pool(name="consts", bufs=1))
    xpool = ctx.enter_context(tc.tile_pool(name="xpool", bufs=2))
    big = ctx.enter_context(tc.tile_pool(name="big", bufs=1))
    atmp = ctx.enter_context(tc.tile_pool(name="atmp", bufs=4))
    state = ctx.enter_context(tc.tile_pool(name="state", bufs=3))
    ostage = ctx.enter_context(tc.tile_pool(name="ostage", bufs=4))
    psA = ctx.enter_context(tc.tile_pool(name="psA", bufs=2, space="PSUM"))
    psTr = ctx.enter_context(tc.tile_pool(name="psTr", bufs=2, space="PSUM"))
    psB = ctx.enter_context(tc.tile_pool(name="psB", bufs=2, space="PSUM"))
    psO = ctx.enter_context(tc.tile_pool(name="psO", bufs=2, space="PSUM"))

    # ---- constants to SBUF ----
    wq_sb = consts.tile([128, KT, C], BF)
    wk_sb = consts.tile([128, KT, C], BF)
    wv_sb = consts.tile([128, KT, C], BF)
    owT_sb = consts.tile([128, CT, V], BF)
    maskT_sb = consts.tile([128, CT, L], FP32)
    crossb_sb = consts.tile([128, PCH], FP32)
    kscale_sb = consts.tile([128, 2], FP32)
    ident_sb = consts.tile([128, 128], BF)
    dident_sb = consts.tile([128, 128], BF)

    nc.sync.dma_start(out=wq_sb, in_=wq_d.ap().rearrange("(a p) c -> p a c", p=128))
    nc.sync.dma_start(out=wk_sb, in_=wk_d.ap().rearrange("(a p) c -> p a c", p=128))
    nc.sync.dma_start(out=wv_sb, in_=wv_d.ap().rearrange("(a p) c -> p a c", p=128))
    nc.sync.dma_start(out=owT_sb, in_=owT_d.ap().rearrange("(a p) v -> p a v", p=128))
    nc.sync.dma_start(out=maskT_sb, in_=maskT_d.ap().rearrange("(a p) i -> p a i", p=128))
    nc.sync.dma_start(out=crossb_sb, in_=crossb_d.ap())
    nc.sync.dma_start(out=kscale_sb, in_=kscale_d.ap())
    nc.sync.dma_start(out=ident_sb, in_=ident_d.ap())
    nc.sync.dma_start(out=dident_sb, in_=dident_d.ap())

    # ---- persistent activations ----
    qT_sb = big.tile([128, CT, T_LOC], BF)    # [c, t] main only
    qtT_sb = big.tile([128, CT, T_LOC], BF)   # scaled q~T
    kT_sb = big.tile([128, CT, T_LOC], BF)    # main only
    ktil_sb = big.tile([128, TTILES, C], BF)  # k~ normal layout, all of T_EXT
    v_sb = big.tile([128, TTILES, C], BF)     # v normal layout, all of T_EXT
    rT_sb = big.tile([128, CT, T_LOC], BF)    # retrieved^T

    xT_ap = xT_d.ap()

    # ---- phase 1: projections, reverse t-chunk order (halo first) ----
    for pch in range(N_PCH - 1, -1, -1):
        t0 = pch * PCH
        is_halo = pch >= N_MAIN_PCH
        xt = xpool.tile([128, KT, PCH], BF, tag="xt")
        nc.sync.dma_start(
            out=xt, in_=xT_ap[:, t0:t0 + PCH].rearrange("(a p) t -> p a t", p=128))

        if not is_halo:
            # qT, kT: [C, t-chunk] via lhsT=w tiles, rhs=xT
            for w_sb, dst, scaled in ((wk_sb, kT_sb, False), (wq_sb, qT_sb, True)):
                for ct in range(CT):
                    ps = psA.tile([128, PCH], FP32, tag="ps")
                    for kt in range(KT):
                        nc.tensor.matmul(
                            ps, lhsT=w_sb[:, kt, ct * 128:(ct + 1) * 128],
                            rhs=xt[:, kt, :],
                            start=(kt == 0), stop=(kt == KT - 1))
                    nc.vector.tensor_copy(dst[:, ct, t0:t0 + PCH], ps)
                    if scaled:
                        nc.vector.tensor_mul(
                            qtT_sb[:, ct, t0:t0 + PCH], ps, crossb_sb)
            # v: normal layout via lhsT=xT tiles, rhs=wv
            for tb in range(PCH // 128):
                tt = t0 // 128 + tb
                ps = psA.tile([128, C], FP32, tag="ps")
                for kt in range(KT):
                    nc.tensor.matmul(
                        ps, lhsT=xt[:, kt, tb * 128:(tb + 1) * 128],
                        rhs=wv_sb[:, kt, :],
                        start=(kt == 0), stop=(kt == KT - 1))
                nc.vector.tensor_copy(v_sb[:, tt, :], ps)
            # k~ via PE transpose of kT with per-position decay row scale
            for tb in range(PCH // 128):
                tt = t0 // 128 + tb
                for ct in range(CT):
                    pst = psTr.tile([128, 128], BF, tag="pst")
                    nc.tensor.transpose(
                        pst, kT_sb[:, ct, t0 + tb * 128:t0 + (tb + 1) * 128],
                        ident_sb)
                    nc.vector.tensor_scalar_mul(
                        ktil_sb[:, tt, ct * 128:(ct + 1) * 128], pst,
                        kscale_sb[:, (tt % 2):(tt % 2) + 1])
        else:
            # halo: only k~ (direct normal projection, scaled) and v
            for tb in range(PCH // 128):
                tt = t0 // 128 + tb
                psk = psA.tile([128, C], FP32, tag="ps")
                for kt in range(KT):
                    nc.tensor.matmul(
                        psk, lhsT=xt[:, kt, tb * 128:(tb + 1) * 128],
                        rhs=wk_sb[:, kt, :],
                        start=(kt == 0), stop=(kt == KT - 1))
                nc.vector.tensor_scalar_mul(
                    ktil_sb[:, tt, :], psk,
                    kscale_sb[:, (tt % 2):(tt % 2) + 1])
                psv = psA.tile([128, C], FP32, tag="ps")
                for kt in range(KT):
                    nc.tensor.matmul(
                        psv, lhsT=xt[:, kt, tb * 128:(tb + 1) * 128],
                        rhs=wv_sb[:, kt, :],
                        start=(kt == 0), stop=(kt == KT - 1))
                nc.vector.tensor_copy(v_sb[:, tt, :], psv)

    # ---- phase 2+3 interleaved: backward retention chunks + output proj ----
    S_cur = None

    def retention_chunk(c):
        nonlocal S_cur
        is_main = c < N_MAIN_CH
        c0 = c * L
        tt0 = c0 // 128  # first t-tile of this chunk (2 per chunk)
        if is_main:
            # AT[j, i] = sum_c' k[j,c'] q[i,c'] ; masked -> atm (bf16)
            atm = []
            for jt in range(2):
                ps = psB.tile([128, L], FP32, tag="ps")
                for ct in range(CT):
                    nc.tensor.matmul(
                        ps, lhsT=kT_sb[:, ct, c0 + jt * 128:c0 + (jt + 1) * 128],
                        rhs=qT_sb[:, ct, c0:c0 + L],
                        start=(ct == 0), stop=(ct == CT - 1))
                am = atmp.tile([128, L], BF, tag="atm")
                nc.vector.tensor_mul(am, ps, maskT_sb[:, jt, :])
                atm.append(am)
            # rT[:, chunk] = v^T @ atm + S^T @ q~T
            for ct in range(CT):
                ps = psB.tile([128, L], FP32, tag="ps")
                for jt in range(2):
                    nc.tensor.matmul(
                        ps, lhsT=v_sb[:, tt0 + jt, ct * 128:(ct + 1) * 128],
                        rhs=atm[jt], start=(jt == 0), stop=False)
                for st in range(CT):
                    nc.tensor.matmul(
                        ps, lhsT=S_cur[:, st, ct * 128:(ct + 1) * 128],
                        rhs=qtT_sb[:, st, c0:c0 + L],
                        start=False, stop=(st == CT - 1))
                nc.vector.tensor_copy(rT_sb[:, ct, c0:c0 + L], ps)
        # state update: S_new = k~^T v + decayL * S_old  (all in PSUM)
        S_new = state.tile([128, CT, C], BF, tag="S")
        for st in range(CT):
            ps = psB.tile([128, C], FP32, tag="ps")
            for jt in range(2):
                nc.tensor.matmul(
                    ps, lhsT=ktil_sb[:, tt0 + jt, st * 128:(st + 1) * 128],
                    rhs=v_sb[:, tt0 + jt, :],
                    start=(jt == 0), stop=(S_cur is None and jt == 1))
            if S_cur is not None:
                nc.tensor.matmul(ps, lhsT=dident_sb, rhs=S_cur[:, st, :],
                                 start=False, stop=True)
            nc.vector.tensor_copy(S_new[:, st, :], ps)
        S_cur = S_new

    def outproj_batch(b):
        # out[v, t] for t-batch b: owT^T @ rT
        t0 = b * PCH
        for vt in range(KT):
            ps = psO.tile([128, PCH], FP32, tag="po")
            for ct in range(CT):
                nc.tensor.matmul(
                    ps, lhsT=owT_sb[:, ct, vt * 128:(vt + 1) * 128],
                    rhs=rT_sb[:, ct, t0:t0 + PCH],
                    start=(ct == 0), stop=(ct == CT - 1))
            ot = ostage.tile([128, PCH], BF, tag="ot")
            nc.vector.tensor_copy(ot, ps)
            nc.sync.dma_start(
                out=outT_d.ap()[vt * 128:(vt + 1) * 128, t0:t0 + PCH], in_=ot)

    for c in range(NCH - 1, -1, -1):
        retention_chunk(c)
        if c < N_MAIN_CH and c % 2 == 0:
            outproj_batch(c // 2)

    ctx.close()


# ---------------- host side ----------------

_NC_CACHE = None


def _get_nc():
    global _NC_CACHE
    if _NC_CACHE is None:
        _NC_CACHE = build_nc()
    return _NC_CACHE


def _prep_in_maps(inputs):
    x = np.asarray(inputs["x"], np.float32)
    basis = np.asarray(inputs["basis"], np.float32)
    decay = float(1.0 / (1.0 + np.exp(-np.float64(inputs["decay_logit"]))))
    out_scale = float(np.float32(inputs["out_scale"]))

    wq = (basis @ np.asarray(inputs["q_coeffs"], np.float32).T).astype(BF16)
    wk = (basis @ np.asarray(inputs["k_coeffs"], np.float32).T).astype(BF16)
    wv = (basis @ np.asarray(inputs["v_coeffs"], np.float32).T).astype(BF16)
    ow = basis @ np.asarray(inputs["o_coeffs"], np.float32).T
    owT = np.ascontiguousarray((ow * out_scale).T).astype(BF16)   # [C, V]

    i = np.arange(L)
    jj, ii = np.meshgrid(i, i, indexing="ij")
    maskT = np.where(jj > ii, decay ** np.maximum(jj - ii - 1, 0), 0.0).astype(np.float32)
    cross = (decay ** (L - 1 - i)).astype(np.float32)               # [L]
    crossb = np.broadcast_to(np.tile(cross, PCH // L)[None, :], (128, PCH))
    crossb = np.ascontiguousarray(crossb, np.float32)
    ksc = decay ** np.arange(2 * 128, dtype=np.float64)
    kscale = np.stack([ksc[:128], ksc[128:]], axis=1).astype(np.float32)  # [128, 2]
    ident = np.eye(128, dtype=np.float32).astype(BF16)
    dident = (np.eye(128, dtype=np.float64) * decay ** L).astype(np.float32).astype(BF16)

    in_maps = []
    for core in range(N_CORES):
        b, h = divmod(core, 2)
        t0 = h * T_LOC
        te = min(t0 + T_EXT, T)
        xT = np.zeros((V, T_EXT), dtype=BF16)
        xT[:, :te - t0] = x[b, t0:te].T.astype(BF16)
        in_maps.append({
            "xT": xT, "wq": wq, "wk": wk, "wv": wv, "owT": owT,
            "maskT": maskT, "crossb": crossb, "kscale": kscale,
            "ident": ident, "dident": dident,
        })
    return in_maps


def _ensure_ntff_hook():
    """The agent image's antenv package lacks axon_hooks; shim it so
    run_bass_kernel_spmd(trace=True) can register the NTFF profile hook."""
    try:
        from antenv.axon_hooks import get_axon_ntff_profile_hook  # noqa: F401
        return
    except ImportError:
        pass
    import sys
    import types
    import antenv
    mod = types.ModuleType("antenv.axon_hooks")
    _state = {"hook": None}
    mod.set_axon_ntff_profile_hook = lambda h: _state.__setitem__("hook", h)
    mod.get_axon_ntff_profile_hook = lambda: _state["hook"]
    sys.modules["antenv.axon_hooks"] = mod
    antenv.axon_hooks = mod
    from trn_agent_boot.trn_boot import _ntff_profile_via_ctypes
    mod.set_axon_ntff_profile_hook(
        _ntff_profile_via_ctypes("/opt/axon/libaxon_pjrt.so"))


def run(inputs, trace=False):
    """Returns (out [B,T,V] float32, BassKernelResults)."""
    if trace:
        _ensure_ntff_hook()
    in_maps = _prep_in_maps(inputs)
    nc = _get_nc()
    res = run_bass_kernel_spmd(nc, in_maps, core_ids=list(range(N_CORES)),
                               trace=trace)
    out = np.zeros((B, T, V), np.float32)
    for core in range(N_CORES):
        b, h = divmod(core, 2)
        outT = np.asarray(res.results[core]["outT"]).astype(np.float32)
        out[b, h * T_LOC:(h + 1) * T_LOC] = outT.T
    return out, res


def kernel(**inputs):
    out, _ = run(inputs, trace=False)
    return out
